# revision 22
# baseline (speedup 1.0000x reference)
"""MoE (8 experts, top-2, SwiGLU FFN) Trainium2 Bass kernel, expert-parallel over 8 cores.

v4 — wide-row home-table routing, contiguous sends, 10-chunk pipelined return
A2A with scatter-add directly into the output.

Strategy (core e owns expert e; core e is also "home" for tokens [512e, 512e+512)):
  - Gate: bf16 hi/lo-split matmul for own TH=512 tokens, top-2 + softmax.
  - Routing: per-token within-home position q per expert via free-dim scan
    matmuls (posi8). Each home scatters 1024 entries (full 256B rows:
    [gid+1 hi, lo, w, rank, 0...]) into its home-table at row
    160*sel + 10*(q%16) + q//16. The whole [1280, 64] table is AllToAll'd
    (expert-major blocks), so core e receives all homes' rows for expert e
    at core-independent offsets. All table touches move full 256B rows ->
    ~128 large DMA descriptors instead of 1280 tiny ones.
  - Expert slot order s = 128*(q//16) + 16*h + (q%16): A2A chunk k covers
    slots [128k, 128k+128) = q in [16k,16k+16) for all homes; the send
    buffer equals ysb row order (plain DMA, no scatter, no zeroing);
    reload tiles are contiguous-span DMAs.
  - FFN: 5 groups of 256 slots; x rows dma_gathered from replicated xbf;
    bf16 matmuls; y = (g@W2 + b2) * w(slot).
  - Return: per chunk AllToAll [128, D] bf16 -> recv; home uses precomputed
    per-chunk scatter indices (from its own table, local) and
    dma_scatter_adds recv rows straight into the bf16 output in ONE call
    (all pool DMAs on queue 0 -> ring-ordered RMW; empty rows go to a dump
    row). Output rows pre-zeroed; b2 added expert-side.
"""

import sys

sys.path.insert(0, "/opt/trn_rl_repo")

import numpy as np
import ml_dtypes

import concourse.bass as bass
import concourse.bacc as bacc
import concourse.mybir as mybir
import concourse.tile as tile

E, TOPK, D, H = 8, 2, 1024, 2048
T = 4096            # total tokens
NCORES = 8
TH = T // NCORES    # home tokens per core = 512
CAP = 160           # per (expert, home) capacity (max observed 153)
C = E * CAP         # compact slots per expert = 1280
NCHUNK = 10         # A2A chunks of 128 slots (q-range 16)
CHL = C // NCHUNK   # 128
NJ = CAP // 16      # 10 j-values (q//16)
HTROWS = C + 2 * TH + 8   # home-table rows: C + dump region
ODUMP = 8           # extra dump rows on the output

BF16 = mybir.dt.bfloat16
F32 = mybir.dt.float32
I16 = mybir.dt.int16
AF = mybir.ActivationFunctionType
OP = mybir.AluOpType

bf16 = ml_dtypes.bfloat16

KD = D // 128    # 8
KH = H // 128    # 16
NCH = TH // 128  # 4 home chunks of own tokens
GL = 256         # FFN group length (slots per group) = 2 A2A chunks
NG = C // GL     # 5 FFN groups


def build_program():
    nc = bacc.Bacc(
        "TRN2",
        target_bir_lowering=False,
        debug=False,
        enable_asserts=True,
        num_devices=NCORES,
        num_swdge_queues=4,
    )

    # ---- per-core inputs ----
    xbf = nc.dram_tensor("xbf", [T, D], BF16, kind="ExternalInput")
    xthi = nc.dram_tensor("xthi", [D, TH], BF16, kind="ExternalInput")
    xtlo = nc.dram_tensor("xtlo", [D, TH], BF16, kind="ExternalInput")
    gwhi = nc.dram_tensor("gwhi", [D, E], BF16, kind="ExternalInput")
    gwlo = nc.dram_tensor("gwlo", [D, E], BF16, kind="ExternalInput")
    w0 = nc.dram_tensor("w0", [D, H], BF16, kind="ExternalInput")
    w1 = nc.dram_tensor("w1", [D, H], BF16, kind="ExternalInput")
    w2 = nc.dram_tensor("w2", [H, D], BF16, kind="ExternalInput")
    b0d = nc.dram_tensor("b0", [H], F32, kind="ExternalInput")
    b1d = nc.dram_tensor("b1", [H], F32, kind="ExternalInput")
    b2d = nc.dram_tensor("b2", [D], F32, kind="ExternalInput")
    # constants
    ltrid = nc.dram_tensor("ltri", [128, 128], BF16, kind="ExternalInput")  # k<=m
    eqr128d = nc.dram_tensor("eqr128", [128, 128], BF16, kind="ExternalInput")
    eqr16d = nc.dram_tensor("eqr16", [16, 128], BF16, kind="ExternalInput")
    wselld = nc.dram_tensor("wsell", [16, E, 128], BF16, kind="ExternalInput")
    eqv8d = nc.dram_tensor("eqv8", [128, E], F32, kind="ExternalInput")
    iota8d = nc.dram_tensor("iota8", [128, E], F32, kind="ExternalInput")
    d127d = nc.dram_tensor("d127", [128, 1], F32, kind="ExternalInput")
    ones1d = nc.dram_tensor("ones1", [1, 128], F32, kind="ExternalInput")
    ghicd = nc.dram_tensor("ghic", [128, 2 * NCH], F32, kind="ExternalInput")
    glocd = nc.dram_tensor("gloc", [128, 2 * NCH], F32, kind="ExternalInput")
    dumpcd = nc.dram_tensor("dumpc", [128, 2 * NCH], F32, kind="ExternalInput")
    eoffd = nc.dram_tensor("eoff", [128, 1], F32, kind="ExternalInput")  # -1-512*core-TH

    out = nc.dram_tensor("out", [TH + ODUMP, D], BF16, kind="ExternalOutput")

    with tile.TileContext(nc) as tc:
        with (
            tc.tile_pool(name="wpool", bufs=1) as wpool,
            tc.tile_pool(name="xg", bufs=1) as xgpool,
            tc.tile_pool(name="gtp", bufs=2) as gtpool,
            tc.tile_pool(name="ysb", bufs=2) as ypool,
            tc.tile_pool(name="rcv", bufs=2) as rcvpool,
            tc.tile_pool(name="consts", bufs=1) as consts,
            tc.tile_pool(name="rt", bufs=1) as rt,
            tc.tile_pool(name="work", bufs=2) as work,
            tc.tile_pool(name="rl", bufs=2) as rlpool,
            tc.tile_pool(name="ps", bufs=6, space="PSUM") as ps,
            tc.tile_pool(name="psg", bufs=2, space="PSUM") as psg_pool,
            tc.tile_pool(name="dram", bufs=1, space="DRAM") as dram,
        ):
            # ---------- DRAM intermediates ----------
            htab = dram.tile([HTROWS, 64], F32)      # scatter target (256B rows)
            htab_x = dram.tile([C, 64], F32)         # post-A2A: my expert's rows
            send = dram.tile([C, D], BF16)
            recvs = [
                dram.tile([CHL, D], BF16, name=f"recv{k}")
                for k in range(NCHUNK)
            ]

            # ---------- gate-critical loads first (SP ring) ----------
            xhisb = consts.tile([128, KD, TH], BF16, tag="xhi")
            xlosb = consts.tile([128, KD, TH], BF16, tag="xlo")
            nc.sync.dma_start(xhisb[:], xthi.ap().rearrange("(k p) t -> p k t", p=128))
            nc.sync.dma_start(xlosb[:], xtlo.ap().rearrange("(k p) t -> p k t", p=128))
            gwhisb = consts.tile([128, KD, E], BF16, tag="gwhi")
            gwlosb = consts.tile([128, KD, E], BF16, tag="gwlo")
            nc.sync.dma_start(gwhisb[:], gwhi.ap().rearrange("(k p) e -> p k e", p=128))
            nc.sync.dma_start(gwlosb[:], gwlo.ap().rearrange("(k p) e -> p k e", p=128))
            # ---------- bulk weight loads (SP ring, behind gate loads) ----------
            w1sb = wpool.tile([128, KD, H], BF16)
            w0sb = wpool.tile([128, KD, H], BF16)
            w2sb = wpool.tile([128, KH, D], BF16)
            nc.sync.dma_start(w1sb[:], w1.ap().rearrange("(k p) h -> p k h", p=128))
            nc.sync.dma_start(w0sb[:], w0.ap().rearrange("(k p) h -> p k h", p=128))
            nc.sync.dma_start(w2sb[:], w2.ap().rearrange("(k p) d -> p k d", p=128))

            # ---------- small consts (ACT ring) ----------
            iota8 = consts.tile([128, E], F32, tag="iota8")
            nc.scalar.dma_start(iota8[:], iota8d.ap())
            ltri = consts.tile([128, 128], BF16, tag="ltri")
            nc.scalar.dma_start(ltri[:], ltrid.ap())
            eqr128 = consts.tile([128, 128], BF16, tag="eqr128")
            nc.scalar.dma_start(eqr128[:], eqr128d.ap())
            eqr16 = consts.tile([16, 128], BF16, tag="eqr16")
            nc.scalar.dma_start(eqr16[:], eqr16d.ap())
            wsell = consts.tile([16, E, 128], BF16, tag="wsell")
            nc.scalar.dma_start(wsell[:], wselld.ap())
            eqv8 = consts.tile([128, E], F32, tag="eqv8")
            nc.scalar.dma_start(eqv8[:], eqv8d.ap())
            d127 = consts.tile([128, 1], F32, tag="d127")
            nc.scalar.dma_start(d127[:], d127d.ap())
            ones1 = consts.tile([1, 128], F32, tag="ones1")
            nc.scalar.dma_start(ones1[:], ones1d.ap())
            ghic = consts.tile([128, 2 * NCH], F32, tag="ghic")
            nc.scalar.dma_start(ghic[:], ghicd.ap())
            gloc = consts.tile([128, 2 * NCH], F32, tag="gloc")
            nc.scalar.dma_start(gloc[:], glocd.ap())
            dumpc = consts.tile([128, 2 * NCH], F32, tag="dumpc")
            nc.scalar.dma_start(dumpc[:], dumpcd.ap())
            eoff = consts.tile([128, 1], F32, tag="eoff")
            nc.scalar.dma_start(eoff[:], eoffd.ap())
            b0sb = consts.tile([128, KH], F32, tag="b0")
            b1sb = consts.tile([128, KH], F32, tag="b1")
            nc.scalar.dma_start(b0sb[:], b0d.ap().rearrange("(h p) -> p h", p=128))
            nc.scalar.dma_start(b1sb[:], b1d.ap().rearrange("(h p) -> p h", p=128))
            b2bc = consts.tile([128, D], F32, tag="b2bc")
            nc.scalar.dma_start(b2bc[:], bass.AP(b2d, 0, [[0, 128], [1, D]]))

            # zero htab rows [0, C) (contiguous: 10 rows x 256B per partition)
            ztg = consts.tile([128, NJ * 64], F32, tag="zgl")
            nc.vector.memset(ztg[:], 0.0)
            nc.scalar.dma_start(
                htab[:C, :].rearrange("(p a) f -> p (a f)", p=128), ztg[:]
            )
            # zero output rows [0, TH)
            ztb = consts.tile([128, D], BF16, tag="zout")
            nc.vector.memset(ztb[:], 0.0)
            outv = out.ap()[:TH].rearrange("(c p) d -> p c d", p=128)
            for cth in range(NCH):
                nc.scalar.dma_start(outv[:, cth, :], ztb[:])

            # ---------- gate: top-2 + softmax over own TH tokens ----------
            rout_sb = rt.tile([128, NCH, 4], F32, tag="routsb")
            eq1sb = rt.tile([128, NCH, E], F32, tag="eq1sb")
            eq2sb = rt.tile([128, NCH, E], F32, tag="eq2sb")
            for c in range(NCH):
                lg = ps.tile([128, E], F32, tag="ps")
                tsl = slice(128 * c, 128 * (c + 1))
                mmi = 0
                for xs, gs in ((xhisb, gwhisb), (xhisb, gwlosb), (xlosb, gwhisb)):
                    for k in range(KD):
                        nc.tensor.matmul(
                            lg[:], xs[:, k, tsl], gs[:, k, :],
                            start=(mmi == 0), stop=(mmi == 3 * KD - 1),
                        )
                        mmi += 1
                m1 = work.tile([128, 1], F32, tag="m1")
                nc.vector.reduce_max(m1[:], lg[:], axis=mybir.AxisListType.X)
                eq1 = eq1sb[:, c, :]
                nc.vector.tensor_scalar(eq1, lg[:], m1[:], None, op0=OP.is_equal)
                msk = work.tile([128, E], F32, tag="msk")
                nc.vector.scalar_tensor_tensor(
                    msk[:], eq1, -1e30, lg[:], op0=OP.mult, op1=OP.add
                )
                m2 = work.tile([128, 1], F32, tag="m2")
                nc.vector.reduce_max(m2[:], msk[:], axis=mybir.AxisListType.X)
                eq2 = eq2sb[:, c, :]
                nc.vector.tensor_scalar(eq2, msk[:], m2[:], None, op0=OP.is_equal)
                t1 = work.tile([128, E], F32, tag="t1")
                nc.vector.tensor_tensor(t1[:], eq1, iota8[:], op=OP.mult)
                nc.vector.reduce_sum(rout_sb[:, c, 0:1], t1[:], axis=mybir.AxisListType.X)
                t2 = work.tile([128, E], F32, tag="t2")
                nc.vector.tensor_tensor(t2[:], eq2, iota8[:], op=OP.mult)
                nc.vector.reduce_sum(rout_sb[:, c, 1:2], t2[:], axis=mybir.AxisListType.X)
                dt = work.tile([128, 1], F32, tag="dt")
                nc.vector.tensor_tensor(dt[:], m2[:], m1[:], op=OP.subtract)
                nc.scalar.activation(rout_sb[:, c, 2:3], dt[:], AF.Sigmoid, scale=-1.0)
                nc.vector.tensor_scalar(
                    rout_sb[:, c, 3:4], rout_sb[:, c, 2:3], -1.0, 1.0,
                    op0=OP.mult, op1=OP.add,
                )

            # ---------- within-home positions per expert (inclusive counts) ----------
            posi8 = rt.tile([128, NCH, E], F32, tag="posi8")
            carry8 = rt.tile([1, E], F32, tag="carry8")
            nc.vector.memset(carry8[:], 0.0)
            oh8 = rt.tile([128, NCH, E], BF16, tag="oh8")
            nc.vector.tensor_tensor(oh8[:], eq1sb[:], eq2sb[:], op=OP.add)
            for c in range(NCH):
                psI = ps.tile([128, E], F32, tag="ps")
                nc.tensor.matmul(psI[:], ltri[:], oh8[:, c, :], start=True, stop=True)
                psC = ps.tile([128, E], F32, tag="ps")
                nc.tensor.matmul(psC[:], ones1[:], carry8[:], start=True, stop=True)
                nc.vector.tensor_copy(posi8[:, c, :], psI[:])
                nc.vector.tensor_tensor(posi8[:, c, :], posi8[:, c, :], psC[:], op=OP.add)
                if c < NCH - 1:
                    prow8 = ps.tile([1, E], F32, tag="ps")
                    nc.tensor.matmul(prow8[:], d127[:], posi8[:, c, :], start=True, stop=True)
                    nc.vector.tensor_copy(carry8[:], prow8[:])

            # ---------- scatter rows + lanes (batched over j = 4*rk + c) ----------
            # row = 160*sel + 10*(q%16) + q//16
            pall = rt.tile([128, 2, NCH], F32, tag="pall")
            pm = rt.tile([128, NCH, E], F32, tag="pm")
            nc.vector.tensor_tensor(pm[:], posi8[:], eq1sb[:], op=OP.mult)
            nc.vector.reduce_sum(pall[:, 0, :], pm[:], axis=mybir.AxisListType.X)
            nc.vector.tensor_tensor(pm[:], posi8[:], eq2sb[:], op=OP.mult)
            nc.vector.reduce_sum(pall[:, 1, :], pm[:], axis=mybir.AxisListType.X)
            # views [128, 2(rk), NCH] on rout_sb lanes
            selv = rout_sb[:].rearrange("p c f -> p f c")[:, 0:2, :]
            wv = rout_sb[:].rearrange("p c f -> p f c")[:, 2:4, :]
            qv8 = rt.tile([128, 2 * NCH], F32, tag="qv8")
            nc.vector.tensor_scalar(qv8[:], pall[:], -1.0, None, op0=OP.add)
            qi8 = work.tile([128, 2 * NCH], I16, tag="qi8")
            nc.vector.tensor_copy(qi8[:], qv8[:])
            qlo8i = work.tile([128, 2 * NCH], I16, tag="qlo8i")
            nc.vector.tensor_scalar(qlo8i[:], qi8[:], 15, None, op0=OP.bitwise_and)
            qlo8 = rt.tile([128, 2 * NCH], F32, tag="qlo8")
            nc.vector.tensor_copy(qlo8[:], qlo8i[:])
            qhi8 = rt.tile([128, 2 * NCH], F32, tag="qhi8")
            nc.vector.tensor_tensor(qhi8[:], qv8[:], qlo8[:], op=OP.subtract)
            nc.vector.tensor_scalar(qhi8[:], qhi8[:], 1.0 / 16.0, None, op0=OP.mult)
            srow = rt.tile([128, 2, NCH], F32, tag="srow")
            nc.vector.tensor_scalar(srow[:], selv, 160.0, None, op0=OP.mult)
            sr2 = rt.tile([128, 2 * NCH], F32, tag="sr2")
            nc.vector.scalar_tensor_tensor(
                sr2[:], qlo8[:], 10.0, srow[:].rearrange("p a b -> p (a b)"),
                op0=OP.mult, op1=OP.add,
            )
            nc.vector.tensor_tensor(sr2[:], sr2[:], qhi8[:], op=OP.add)
            # overflow guard: q >= CAP -> dump row
            ge8 = work.tile([128, 2 * NCH], F32, tag="ge8")
            nc.vector.tensor_scalar(ge8[:], qv8[:], float(CAP), None, op0=OP.is_ge)
            dd8 = work.tile([128, 2 * NCH], F32, tag="dd8")
            nc.vector.tensor_tensor(dd8[:], dumpc[:], sr2[:], op=OP.subtract)
            nc.vector.tensor_tensor(dd8[:], dd8[:], ge8[:], op=OP.mult)
            nc.vector.tensor_tensor(sr2[:], sr2[:], dd8[:], op=OP.add)
            # lanes [128, 8, 64]: [ghi, glo, w, rank, 0...]
            lanes = rt.tile([128, 2 * NCH, 64], F32, tag="lanes")
            nc.vector.memset(lanes[:], 0.0)
            nc.vector.tensor_copy(lanes[:, :, 0], ghic[:])
            nc.vector.tensor_copy(lanes[:, :, 1], gloc[:])
            nc.vector.tensor_copy(
                lanes[:, :, 2].rearrange("p (a b) -> p a b", a=2), wv
            )
            nc.vector.memset(lanes[:, 0:NCH, 3], 1.0)
            nc.vector.memset(lanes[:, NCH:, 3], 2.0)

            # wrap16 of srow over entries i = 128j + p
            shi = rt.tile([128, 2 * NCH], F32, tag="shi")
            slo = rt.tile([128, 2 * NCH], F32, tag="slo")
            svi = work.tile([128, 2 * NCH], I16, tag="svi")
            nc.vector.tensor_copy(svi[:], sr2[:])
            sloi = work.tile([128, 2 * NCH], I16, tag="sloi")
            nc.vector.tensor_scalar(sloi[:], svi[:], 31, None, op0=OP.bitwise_and)
            nc.vector.tensor_copy(slo[:], sloi[:])
            nc.vector.tensor_tensor(shi[:], sr2[:], slo[:], op=OP.subtract)
            nc.vector.tensor_scalar(shi[:], shi[:], 1.0 / 32.0, None, op0=OP.mult)
            rrhs = rt.tile([128, 2, 2 * NCH, E], BF16, tag="rrhs")
            for v in range(E):
                nc.vector.tensor_scalar(
                    rrhs[:, 0, :, v], shi[:], eqv8[:, v : v + 1], None, op0=OP.mult
                )
                nc.vector.tensor_scalar(
                    rrhs[:, 1, :, v], slo[:], eqv8[:, v : v + 1], None, op0=OP.mult
                )
            psr = ps.tile([128, 2, 2 * NCH * E], F32, tag="ps")
            nc.tensor.matmul(
                psr[:].rearrange("p a b -> p (a b)"),
                eqr128[:],
                rrhs[:].rearrange("p a b c -> p (a b c)"),
                start=True, stop=True,
            )
            sidxf = rt.tile([128, 2 * NCH * E], F32, tag="sidxf")
            nc.vector.tensor_scalar(sidxf[:], psr[:, 0, :], 32.0, None, op0=OP.mult)
            nc.vector.tensor_tensor(sidxf[:], sidxf[:], psr[:, 1, :], op=OP.add)
            sidx = rt.tile([128, 2 * NCH * E], I16, tag="sidx")
            nc.vector.tensor_copy(sidx[:], sidxf[:])

            # scatter 1024 full-row entries into the home table
            nc.gpsimd.dma_scatter_add(
                htab[:, :], lanes[:], sidx[:],
                num_idxs=2 * TH, num_idxs_reg=2 * TH, elem_size=64, elem_step=64,
                queue_num=0,
            )

            # ---------- home-side return reloads + per-chunk scatter indices ----------
            # (local: only needs htab; runs while the table A2A is in flight)
            ridxs = []
            for k in range(NCHUNK):
                rlt = rlpool.tile([16, E, 64], F32, tag="rlt")
                nc.scalar.dma_start(
                    rlt[:],
                    htab[:C, :].rearrange("(e u j) l -> j u e l", u=16, j=NJ)[k],
                )
                rgr = rlpool.tile([16, 3, E], BF16, tag="rgr")
                nc.vector.tensor_copy(rgr[:, 0, :], rlt[:, :, 0])
                nc.vector.tensor_copy(rgr[:, 1, :], rlt[:, :, 1])
                nc.vector.tensor_copy(rgr[:, 2, :], rlt[:, :, 3])
                psr2 = psg_pool.tile([128, 3, E], F32, tag="psg")
                nc.tensor.matmul(
                    psr2[:].rearrange("p a b -> p (a b)"),
                    eqr16[:],
                    rgr[:].rearrange("p a b -> p (a b)"),
                    start=True, stop=True,
                )
                idx0 = rlpool.tile([128, E], F32, tag="idx0")
                nc.vector.tensor_scalar(idx0[:], psr2[:, 0, :], 32.0, None, op0=OP.mult)
                nc.vector.tensor_tensor(idx0[:], idx0[:], psr2[:, 1, :], op=OP.add)
                # per-rank: idx = mr*(idx0 - 1 - 512*core - TH) + TH (local row or dump)
                rpair = []
                for r in (1.0, 2.0):
                    mr = rlpool.tile([128, E], F32, tag="mr")
                    nc.vector.tensor_scalar(mr[:], psr2[:, 2, :], r, None, op0=OP.is_equal)
                    nz = rlpool.tile([128, E], F32, tag="nz")
                    nc.vector.tensor_scalar(nz[:], idx0[:], 0.5, None, op0=OP.is_ge)
                    nc.vector.tensor_tensor(mr[:], mr[:], nz[:], op=OP.mult)
                    ridf = rlpool.tile([128, E], F32, tag="ridf")
                    nc.vector.tensor_scalar(ridf[:], idx0[:], eoff[:], None, op0=OP.add)
                    nc.vector.tensor_tensor(ridf[:], ridf[:], mr[:], op=OP.mult)
                    nc.vector.tensor_scalar(ridf[:], ridf[:], float(TH), None, op0=OP.add)
                    ridx = consts.tile([128, E], I16, tag=f"ridx{k}_{int(r)}")
                    nc.vector.tensor_copy(ridx[:], ridf[:])
                    rpair.append(ridx)
                ridxs.append(rpair)

            # ---------- table A2A (expert-major blocks) ----------
            nc.gpsimd.collective_compute(
                "AllToAll", OP.bypass,
                replica_groups=[list(range(NCORES))],
                ins=[htab[:C, :].rearrange("a b -> (a b)")],
                outs=[htab_x[:].rearrange("a b -> (a b)")],
            )

            # ---------- expert-side reload: [16(u), 8(h), 10(j), 64] ----------
            # loaded in two j-halves to halve SBUF; f = 8*j + h so each half
            # covers a contiguous f-range of 40
            grhs = rt.tile([16, 2, 80], BF16, tag="grhs")
            wspl = rt.tile([16, 2, 80], BF16, tag="wspl")
            rtsrc = htab_x[:].rearrange("(h u j) l -> u h j l", u=16, j=NJ)
            for jh in range(2):
                rtile = rt.tile([16, E, NJ // 2, 64], F32, tag="rtile")
                nc.scalar.dma_start(
                    rtile[:], rtsrc[:, :, 5 * jh : 5 * (jh + 1), :]
                )
                rtv = rtile[:].rearrange("u h j l -> u j h l")
                fsl = slice(40 * jh, 40 * (jh + 1))
                nc.vector.tensor_copy(grhs[:, 0, fsl], rtv[:, :, :, 0])
                nc.vector.tensor_copy(grhs[:, 1, fsl], rtv[:, :, :, 1])
                nc.vector.tensor_copy(wspl[:, 0, fsl], rtv[:, :, :, 2])
                wrem = work.tile([16, 40], F32, tag="wrem")
                nc.vector.tensor_tensor(
                    wrem[:], rtv[:, :, :, 2], wspl[:, 0, fsl], op=OP.subtract
                )
                nc.vector.tensor_copy(wspl[:, 1, fsl], wrem[:])
            psg = psg_pool.tile([128, 2, 80], F32, tag="psg")
            nc.tensor.matmul(
                psg[:].rearrange("p a b -> p (a b)"),
                eqr16[:],
                grhs[:].rearrange("p a b -> p (a b)"),
                start=True, stop=True,
            )
            gxf = rt.tile([128, 80], F32, tag="gxf")
            nc.vector.tensor_scalar(gxf[:], psg[:, 0, :], 32.0, None, op0=OP.mult)
            nc.vector.tensor_tensor(gxf[:], gxf[:], psg[:, 1, :], op=OP.add)
            nc.vector.tensor_scalar(gxf[:], gxf[:], -1.0, None, op0=OP.add)
            nc.vector.tensor_scalar(gxf[:], gxf[:], 0.0, None, op0=OP.max)
            gxidx = rt.tile([128, 80], I16, tag="gxidx")
            nc.vector.tensor_copy(gxidx[:], gxf[:])
            # per-slot weight, slot-major [p = s%128, a = s//128], via wsell matmuls
            psw = psg_pool.tile([128, 2, NJ], F32, tag="psg")
            for b in range(E):
                nc.tensor.matmul(
                    psw[:].rearrange("p a b -> p (a b)"),
                    wsell[:, b, :],
                    wspl[:, :, b::8].rearrange("p a b -> p (a b)"),
                    start=(b == 0), stop=(b == E - 1),
                )
            wslot = rt.tile([128, NJ], F32, tag="wslot")
            nc.vector.tensor_copy(wslot[:], psw[:, 0, :])
            nc.vector.tensor_tensor(wslot[:], wslot[:], psw[:, 1, :], op=OP.add)

            # ---------- x-gathers: 3-buffer ring, 3 prefetched upfront ----------
            def gather_x(g):
                xtf = xgpool.tile([128, KD, GL], BF16, tag=f"xtg{g % 3}")
                nc.gpsimd.dma_gather(
                    xtf[:], xbf.ap(),
                    gxidx[:, 16 * g : 16 * (g + 1)],
                    num_idxs=GL, num_idxs_reg=GL, elem_size=D, transpose=True,
                    queue_num=0,
                )
                return xtf

            xtfs = {g: gather_x(g) for g in range(3)}

            # ---------- FFN over 5 groups of 256 slots + pipelined return ----------
            for g in range(NG):
                xtf = xtfs.pop(g)
                if g + 3 <= NG - 1:
                    xtfs[g + 3] = gather_x(g + 3)
                gt = gtpool.tile([128, KH, GL], BF16, tag="gt")
                for h in range(KH):
                    ph1 = ps.tile([128, GL], F32, tag="ps")
                    for k in range(KD):
                        nc.tensor.matmul(
                            ph1[:], w1sb[:, k, 128 * h : 128 * (h + 1)],
                            xtf[:, k, :],
                            start=(k == 0), stop=(k == KD - 1),
                        )
                    ph0 = ps.tile([128, GL], F32, tag="ps")
                    for k in range(KD):
                        nc.tensor.matmul(
                            ph0[:], w0sb[:, k, 128 * h : 128 * (h + 1)],
                            xtf[:, k, :],
                            start=(k == 0), stop=(k == KD - 1),
                        )
                    sig = work.tile([128, GL], F32, tag="sig")
                    nc.scalar.activation(
                        sig[:], ph1[:], AF.Sigmoid, bias=b1sb[:, h : h + 1]
                    )
                    zb = work.tile([128, GL], F32, tag="zb")
                    nc.vector.tensor_scalar(
                        zb[:], ph1[:], b1sb[:, h : h + 1], None, op0=OP.add
                    )
                    nc.vector.tensor_tensor(zb[:], zb[:], sig[:], op=OP.mult)
                    nc.vector.scalar_tensor_tensor(
                        gt[:, h, :], ph0[:], b0sb[:, h : h + 1], zb[:],
                        op0=OP.add, op1=OP.mult,
                    )
                ysb = ypool.tile([128, GL // 128, D], BF16, tag="ysbg")
                for c2 in range(GL // 128):
                    a = (GL // 128) * g + c2
                    for n in range(D // 512):
                        py = ps.tile([128, 512], F32, tag="ps")
                        for k in range(KH):
                            nc.tensor.matmul(
                                py[:],
                                gt[:, k, 128 * c2 : 128 * (c2 + 1)],
                                w2sb[:, k, 512 * n : 512 * (n + 1)],
                                start=(k == 0), stop=(k == KH - 1),
                            )
                        yb = work.tile([128, 512], F32, tag="yb")
                        nc.vector.tensor_tensor(
                            yb[:], py[:], b2bc[:, 512 * n : 512 * (n + 1)], op=OP.add
                        )
                        nc.vector.tensor_scalar(
                            ysb[:, c2, 512 * n : 512 * (n + 1)], yb[:],
                            wslot[:, a : a + 1], None, op0=OP.mult,
                        )
                    # per-half: contiguous send + A2A + recv + scatter into out
                    kch = (GL // 128) * g + c2
                    nc.sync.dma_start(
                        send[CHL * kch : CHL * (kch + 1), :].rearrange(
                            "(a p) d -> p a d", p=128
                        ),
                        ysb[:, c2 : c2 + 1, :],
                    )
                    nc.gpsimd.collective_compute(
                        "AllToAll", OP.bypass,
                        replica_groups=[list(range(NCORES))],
                        ins=[
                            send[CHL * kch : CHL * (kch + 1), :].rearrange(
                                "a b -> (a b)"
                            )
                        ],
                        outs=[recvs[kch][:].rearrange("a b -> (a b)")],
                    )
                    recvsb = rcvpool.tile([128, 1, D], BF16, tag="recvsb")
                    nc.sync.dma_start(
                        recvsb[:], recvs[kch][:].rearrange("(a p) d -> p a d", p=128)
                    )
                    for rr in range(2):
                        nc.gpsimd.dma_scatter_add(
                            out.ap(), recvsb[:], ridxs[kch][rr][:],
                            num_idxs=CHL, num_idxs_reg=CHL, elem_size=D,
                            queue_num=0,
                        )

    nc.compile()
    return nc


def _split_bf16(a):
    hi = a.astype(bf16)
    lo = (a - hi.astype(np.float32)).astype(bf16)
    return hi, lo


def make_in_maps(inputs, gate_w, W0, b0, W1, b1, W2, b2):
    x = np.ascontiguousarray(np.asarray(inputs).reshape(-1, D).astype(np.float32))
    xbf = x.astype(bf16)
    gwT = np.ascontiguousarray(np.asarray(gate_w).astype(np.float32).T)  # [D, E]
    gwhi, gwlo = _split_bf16(gwT)

    p = np.arange(128)
    m = np.arange(128)
    ltri = np.triu(np.ones((128, 128), np.float32)).astype(bf16)
    eqr128 = ((p[:, None] % 16) == (m[None, :] % 16)).astype(np.float32).astype(bf16)
    u = np.arange(16)
    eqr16 = (u[:, None] == (m[None, :] % 16)).astype(np.float32).astype(bf16)
    wsell = np.zeros((16, E, 128), np.float32)
    for b in range(E):
        wsell[:, b, :] = (u[:, None] == (m[None, :] % 16)) & (b == (m[None, :] // 16))
    wsell = wsell.astype(bf16)
    eqv8 = ((p[:, None] // 16) == np.arange(E)[None, :]).astype(np.float32)
    iota8 = np.tile(np.arange(E, dtype=np.float32)[None, :], (128, 1))
    d127 = np.zeros((128, 1), np.float32)
    d127[127, 0] = 1.0
    ones1 = np.ones((1, 128), np.float32)
    # dump rows for scatter overflow: j = 4*rk + c
    dumpc = np.zeros((128, 2 * NCH), np.float32)
    for rk in range(2):
        for c in range(NCH):
            dumpc[:, 4 * rk + c] = C + 2 * (128 * c + p) + rk

    W0 = np.asarray(W0)
    W1 = np.asarray(W1)
    W2 = np.asarray(W2)
    b0 = np.asarray(b0)
    b1 = np.asarray(b1)
    b2 = np.asarray(b2)

    in_maps = []
    for e in range(NCORES):
        xT_own = np.ascontiguousarray(x[e * TH : (e + 1) * TH].T)  # [D, TH]
        xthi, xtlo = _split_bf16(xT_own)
        gid1c = 512 * e + 128 * np.arange(NCH)[None, :] + p[:, None] + 1  # [128, NCH]
        gid1 = np.concatenate([gid1c, gid1c], axis=1)  # [128, 8] cols j = 4rk+c
        mm = {
            "xbf": xbf,
            "xthi": xthi,
            "xtlo": xtlo,
            "gwhi": gwhi,
            "gwlo": gwlo,
            "w0": np.ascontiguousarray(W0[e].astype(bf16)),
            "w1": np.ascontiguousarray(W1[e].astype(bf16)),
            "w2": np.ascontiguousarray(W2[e].astype(bf16)),
            "b0": np.ascontiguousarray(b0[e].astype(np.float32)),
            "b1": np.ascontiguousarray(b1[e].astype(np.float32)),
            "b2": np.ascontiguousarray(b2[e].astype(np.float32)),
            "ltri": ltri,
            "eqr128": eqr128,
            "eqr16": eqr16,
            "wsell": wsell,
            "eqv8": eqv8,
            "iota8": iota8,
            "d127": d127,
            "ones1": ones1,
            "ghic": np.ascontiguousarray((gid1 // 32).astype(np.float32)),
            "gloc": np.ascontiguousarray((gid1 % 32).astype(np.float32)),
            "dumpc": dumpc,
            "eoff": np.full((128, 1), -1.0 - 512.0 * e - TH, np.float32),
        }
        in_maps.append(mm)
    return in_maps


_NC_CACHE = {}


def get_program(mode="full"):
    if mode not in _NC_CACHE:
        _NC_CACHE[mode] = build_program()
    return _NC_CACHE[mode]


def kernel(**inputs):
    from concourse.bass_utils import run_bass_kernel_spmd

    nc = get_program()
    in_maps = make_in_maps(**inputs)
    res = run_bass_kernel_spmd(nc, in_maps, core_ids=list(range(NCORES)))
    outs = [
        np.asarray(res.results[c]["out"], dtype=np.float32)[:TH]
        for c in range(NCORES)
    ]
    full = np.concatenate(outs, axis=0)
    return full.reshape(np.asarray(inputs["inputs"]).shape)


# revision 35
# speedup vs baseline: 1.1288x; 1.1288x over previous
"""MoE (8 experts, top-2, SwiGLU FFN) Trainium2 Bass kernel, expert-parallel over 8 cores.

v4 — wide-row home-table routing, contiguous sends, 10-chunk pipelined return
A2A with scatter-add directly into the output.

Strategy (core e owns expert e; core e is also "home" for tokens [512e, 512e+512)):
  - Gate: bf16 hi/lo-split matmul for own TH=512 tokens, top-2 + softmax.
  - Routing: per-token within-home position q per expert via free-dim scan
    matmuls (posi8). Each home scatters 1024 entries (full 256B rows:
    [gid+1 hi, lo, w, rank, 0...]) into its home-table at row
    160*sel + 10*(q%16) + q//16. The whole [1280, 64] table is AllToAll'd
    (expert-major blocks), so core e receives all homes' rows for expert e
    at core-independent offsets. All table touches move full 256B rows ->
    ~128 large DMA descriptors instead of 1280 tiny ones.
  - Expert slot order s = 128*(q//16) + 16*h + (q%16): A2A chunk k covers
    slots [128k, 128k+128) = q in [16k,16k+16) for all homes; the send
    buffer equals ysb row order (plain DMA, no scatter, no zeroing);
    reload tiles are contiguous-span DMAs.
  - FFN: 5 groups of 256 slots; x rows dma_gathered from replicated xbf;
    bf16 matmuls; y = (g@W2 + b2) * w(slot).
  - Return: per chunk AllToAll [128, D] bf16 into a slice of one recv
    [1280, D]; after the last chunk the home gathers its two y rows per
    token (recv row = 128*(q//16) + 16*sel + q%16, indices built from its
    own routing state - no table reload), adds them (bf16) and writes the
    output with one DMA. Gathers are read-only, so no RMW scatter chains.
    b2 added expert-side (combine weights sum to 1).
"""

import sys

sys.path.insert(0, "/opt/trn_rl_repo")

import numpy as np
import ml_dtypes

import concourse.bass as bass
import concourse.bacc as bacc
import concourse.mybir as mybir
import concourse.tile as tile

E, TOPK, D, H = 8, 2, 1024, 2048
T = 4096            # total tokens
NCORES = 8
TH = T // NCORES    # home tokens per core = 512
CAP = 160           # per (expert, home) capacity (max observed 153)
C = E * CAP         # compact slots per expert = 1280
NCHUNK = 10         # A2A chunks of 128 slots (q-range 16)
CHL = C // NCHUNK   # 128
NJ = CAP // 16      # 10 j-values (q//16)
HTROWS = C + 2 * TH + 8   # home-table rows: C + dump region
ODUMP = 8           # extra dump rows on the output

BF16 = mybir.dt.bfloat16
F32 = mybir.dt.float32
I16 = mybir.dt.int16
AF = mybir.ActivationFunctionType
OP = mybir.AluOpType

bf16 = ml_dtypes.bfloat16

KD = D // 128    # 8
KH = H // 128    # 16
NCH = TH // 128  # 4 home chunks of own tokens
GL = 256         # FFN group length (slots per group) = 2 A2A chunks
NG = C // GL     # 5 FFN groups


def build_program():
    nc = bacc.Bacc(
        "TRN2",
        target_bir_lowering=False,
        debug=False,
        enable_asserts=True,
        num_devices=NCORES,
        num_swdge_queues=4,
    )

    # ---- per-core inputs ----
    xbf = nc.dram_tensor("xbf", [T, D], BF16, kind="ExternalInput")
    xthi = nc.dram_tensor("xthi", [D, TH], BF16, kind="ExternalInput")
    xtlo = nc.dram_tensor("xtlo", [D, TH], BF16, kind="ExternalInput")
    gwhi = nc.dram_tensor("gwhi", [D, E], BF16, kind="ExternalInput")
    gwlo = nc.dram_tensor("gwlo", [D, E], BF16, kind="ExternalInput")
    w0 = nc.dram_tensor("w0", [D, H], BF16, kind="ExternalInput")
    w1 = nc.dram_tensor("w1", [D, H], BF16, kind="ExternalInput")
    w2 = nc.dram_tensor("w2", [H, D], BF16, kind="ExternalInput")
    b0d = nc.dram_tensor("b0", [H], F32, kind="ExternalInput")
    b1d = nc.dram_tensor("b1", [H], F32, kind="ExternalInput")
    b2d = nc.dram_tensor("b2", [D], F32, kind="ExternalInput")
    # constants
    ltrid = nc.dram_tensor("ltri", [128, 128], BF16, kind="ExternalInput")  # k<=m
    eqr128d = nc.dram_tensor("eqr128", [128, 128], BF16, kind="ExternalInput")
    eqr16d = nc.dram_tensor("eqr16", [16, 128], BF16, kind="ExternalInput")
    wselld = nc.dram_tensor("wsell", [16, E, 128], BF16, kind="ExternalInput")
    eqv8d = nc.dram_tensor("eqv8", [128, E], F32, kind="ExternalInput")
    iota8d = nc.dram_tensor("iota8", [128, NCH, E], F32, kind="ExternalInput")
    d127d = nc.dram_tensor("d127", [128, 1], F32, kind="ExternalInput")
    ones1d = nc.dram_tensor("ones1", [1, 128], F32, kind="ExternalInput")
    ghicd = nc.dram_tensor("ghic", [128, 2 * NCH], F32, kind="ExternalInput")
    glocd = nc.dram_tensor("gloc", [128, 2 * NCH], F32, kind="ExternalInput")
    dumpcd = nc.dram_tensor("dumpc", [128, 2 * NCH], F32, kind="ExternalInput")

    out = nc.dram_tensor("out", [TH, D], BF16, kind="ExternalOutput")

    with tile.TileContext(nc) as tc:
        with (
            tc.tile_pool(name="wpool", bufs=1) as wpool,
            tc.tile_pool(name="xg", bufs=1) as xgpool,
            tc.tile_pool(name="gtp", bufs=2) as gtpool,
            tc.tile_pool(name="ysb", bufs=2) as ypool,
            tc.tile_pool(name="consts", bufs=1) as consts,
            tc.tile_pool(name="rt", bufs=1) as rt,
            tc.tile_pool(name="work", bufs=2) as work,
            tc.tile_pool(name="ps", bufs=6, space="PSUM") as ps,
            tc.tile_pool(name="psg", bufs=2, space="PSUM") as psg_pool,
            tc.tile_pool(name="dram", bufs=1, space="DRAM") as dram,
        ):
            # ---------- DRAM intermediates ----------
            htab = dram.tile([HTROWS, 64], F32)      # scatter target (256B rows)
            htab_x = dram.tile([C, 64], F32)         # post-A2A: my expert's rows
            send = dram.tile([C, D], BF16)
            recv = dram.tile([C, D], BF16)

            # ---------- gate-critical loads first (SP ring) ----------
            xhisb = consts.tile([128, KD, TH], BF16, tag="xhi")
            xlosb = consts.tile([128, KD, TH], BF16, tag="xlo")
            nc.sync.dma_start(xhisb[:], xthi.ap().rearrange("(k p) t -> p k t", p=128))
            nc.sync.dma_start(xlosb[:], xtlo.ap().rearrange("(k p) t -> p k t", p=128))
            gwhisb = consts.tile([128, KD, E], BF16, tag="gwhi")
            gwlosb = consts.tile([128, KD, E], BF16, tag="gwlo")
            nc.sync.dma_start(gwhisb[:], gwhi.ap().rearrange("(k p) e -> p k e", p=128))
            nc.sync.dma_start(gwlosb[:], gwlo.ap().rearrange("(k p) e -> p k e", p=128))
            # ---------- bulk weight loads (SP ring, behind gate loads) ----------
            w1sb = wpool.tile([128, KD, H], BF16)
            w0sb = wpool.tile([128, KD, H], BF16)
            w2sb = wpool.tile([128, KH, D], BF16)
            nc.sync.dma_start(w1sb[:], w1.ap().rearrange("(k p) h -> p k h", p=128))
            nc.sync.dma_start(w0sb[:], w0.ap().rearrange("(k p) h -> p k h", p=128))
            nc.sync.dma_start(w2sb[:], w2.ap().rearrange("(k p) d -> p k d", p=128))

            # ---------- small consts (ACT ring) ----------
            iota8 = consts.tile([128, NCH, E], F32, tag="iota8")
            nc.scalar.dma_start(iota8[:], iota8d.ap())
            ltri = consts.tile([128, 128], BF16, tag="ltri")
            nc.scalar.dma_start(ltri[:], ltrid.ap())
            eqr128 = consts.tile([128, 128], BF16, tag="eqr128")
            nc.scalar.dma_start(eqr128[:], eqr128d.ap())
            eqr16 = consts.tile([16, 128], BF16, tag="eqr16")
            nc.scalar.dma_start(eqr16[:], eqr16d.ap())
            wsell = consts.tile([16, E, 128], BF16, tag="wsell")
            nc.scalar.dma_start(wsell[:], wselld.ap())
            eqv8 = consts.tile([128, E], F32, tag="eqv8")
            nc.scalar.dma_start(eqv8[:], eqv8d.ap())
            d127 = consts.tile([128, 1], F32, tag="d127")
            nc.scalar.dma_start(d127[:], d127d.ap())
            ones1 = consts.tile([1, 128], F32, tag="ones1")
            nc.scalar.dma_start(ones1[:], ones1d.ap())
            ghic = consts.tile([128, 2 * NCH], F32, tag="ghic")
            nc.scalar.dma_start(ghic[:], ghicd.ap())
            gloc = consts.tile([128, 2 * NCH], F32, tag="gloc")
            nc.scalar.dma_start(gloc[:], glocd.ap())
            dumpc = consts.tile([128, 2 * NCH], F32, tag="dumpc")
            nc.scalar.dma_start(dumpc[:], dumpcd.ap())
            b0sb = consts.tile([128, KH], F32, tag="b0")
            b1sb = consts.tile([128, KH], F32, tag="b1")
            nc.scalar.dma_start(b0sb[:], b0d.ap().rearrange("(h p) -> p h", p=128))
            nc.scalar.dma_start(b1sb[:], b1d.ap().rearrange("(h p) -> p h", p=128))
            b2bc = consts.tile([128, D], F32, tag="b2bc")
            nc.scalar.dma_start(b2bc[:], bass.AP(b2d, 0, [[0, 128], [1, D]]))

            # zero htab rows [0, C) (contiguous: 10 rows x 256B per partition)
            ztg = consts.tile([128, NJ * 64], F32, tag="zgl")
            nc.vector.memset(ztg[:], 0.0)
            nc.scalar.dma_start(
                htab[:C, :].rearrange("(p a) f -> p (a f)", p=128), ztg[:]
            )

            # ---------- gate: top-2 + softmax over own TH tokens ----------
            rout_sb = rt.tile([128, NCH, 4], F32, tag="routsb")
            eq1sb = rt.tile([128, NCH, E], F32, tag="eq1sb")
            eq2sb = rt.tile([128, NCH, E], F32, tag="eq2sb")
            dts = rt.tile([128, NCH], F32, tag="dts")
            for c in range(NCH):
                lg = ps.tile([128, E], F32, tag="ps")
                tsl = slice(128 * c, 128 * (c + 1))
                mmi = 0
                for xs, gs in ((xhisb, gwhisb), (xhisb, gwlosb), (xlosb, gwhisb)):
                    for k in range(KD):
                        nc.tensor.matmul(
                            lg[:], xs[:, k, tsl], gs[:, k, :],
                            start=(mmi == 0), stop=(mmi == 3 * KD - 1),
                        )
                        mmi += 1
                m1 = work.tile([128, 1], F32, tag="m1")
                nc.vector.reduce_max(m1[:], lg[:], axis=mybir.AxisListType.X)
                eq1 = eq1sb[:, c, :]
                nc.vector.tensor_scalar(eq1, lg[:], m1[:], None, op0=OP.is_equal)
                msk = work.tile([128, E], F32, tag="msk")
                nc.vector.scalar_tensor_tensor(
                    msk[:], eq1, -1e30, lg[:], op0=OP.mult, op1=OP.add
                )
                m2 = work.tile([128, 1], F32, tag="m2")
                nc.vector.reduce_max(m2[:], msk[:], axis=mybir.AxisListType.X)
                eq2 = eq2sb[:, c, :]
                nc.vector.tensor_scalar(eq2, msk[:], m2[:], None, op0=OP.is_equal)
                nc.vector.tensor_tensor(
                    dts[:, c : c + 1], m2[:], m1[:], op=OP.subtract
                )
            # batched gate tail: sel1/sel2 + softmax weights
            tq = rt.tile([128, NCH, E], F32, tag="tq")
            for eqs, lane in ((eq1sb, 0), (eq2sb, 1)):
                nc.vector.tensor_tensor(tq[:], eqs[:], iota8[:], op=OP.mult)
                nc.vector.reduce_sum(
                    rout_sb[:].rearrange("p c f -> p f c")[:, lane, :],
                    tq[:], axis=mybir.AxisListType.X,
                )
            wsig = rt.tile([128, NCH], F32, tag="wsig")
            nc.scalar.activation(wsig[:], dts[:], AF.Sigmoid, scale=-1.0)
            nc.vector.tensor_copy(
                rout_sb[:].rearrange("p c f -> p f c")[:, 2, :], wsig[:]
            )
            nc.vector.tensor_scalar(
                rout_sb[:].rearrange("p c f -> p f c")[:, 3, :], wsig[:],
                -1.0, 1.0, op0=OP.mult, op1=OP.add,
            )

            # ---------- within-home positions per expert (inclusive counts) ----------
            posi8 = rt.tile([128, NCH, E], F32, tag="posi8")
            carry8 = rt.tile([1, E], F32, tag="carry8")
            nc.vector.memset(carry8[:], 0.0)
            oh8 = rt.tile([128, NCH, E], BF16, tag="oh8")
            nc.vector.tensor_tensor(oh8[:], eq1sb[:], eq2sb[:], op=OP.add)
            for c in range(NCH):
                psI = ps.tile([128, E], F32, tag="ps")
                nc.tensor.matmul(psI[:], ltri[:], oh8[:, c, :], start=True, stop=True)
                psC = ps.tile([128, E], F32, tag="ps")
                nc.tensor.matmul(psC[:], ones1[:], carry8[:], start=True, stop=True)
                nc.vector.tensor_copy(posi8[:, c, :], psI[:])
                nc.vector.tensor_tensor(posi8[:, c, :], posi8[:, c, :], psC[:], op=OP.add)
                if c < NCH - 1:
                    prow8 = ps.tile([1, E], F32, tag="ps")
                    nc.tensor.matmul(prow8[:], d127[:], posi8[:, c, :], start=True, stop=True)
                    nc.vector.tensor_copy(carry8[:], prow8[:])

            # ---------- scatter rows + lanes (batched over j = 4*rk + c) ----------
            # row = 160*sel + 10*(q%16) + q//16
            pall = rt.tile([128, 2, NCH], F32, tag="pall")
            pm = rt.tile([128, NCH, E], F32, tag="pm")
            nc.vector.tensor_tensor(pm[:], posi8[:], eq1sb[:], op=OP.mult)
            nc.vector.reduce_sum(pall[:, 0, :], pm[:], axis=mybir.AxisListType.X)
            nc.vector.tensor_tensor(pm[:], posi8[:], eq2sb[:], op=OP.mult)
            nc.vector.reduce_sum(pall[:, 1, :], pm[:], axis=mybir.AxisListType.X)
            # views [128, 2(rk), NCH] on rout_sb lanes
            selv = rout_sb[:].rearrange("p c f -> p f c")[:, 0:2, :]
            wv = rout_sb[:].rearrange("p c f -> p f c")[:, 2:4, :]
            qv8 = rt.tile([128, 2 * NCH], F32, tag="qv8")
            nc.vector.tensor_scalar(qv8[:], pall[:], -1.0, None, op0=OP.add)
            qi8 = work.tile([128, 2 * NCH], I16, tag="qi8")
            nc.vector.tensor_copy(qi8[:], qv8[:])
            qlo8i = work.tile([128, 2 * NCH], I16, tag="qlo8i")
            nc.vector.tensor_scalar(qlo8i[:], qi8[:], 15, None, op0=OP.bitwise_and)
            qlo8 = rt.tile([128, 2 * NCH], F32, tag="qlo8")
            nc.vector.tensor_copy(qlo8[:], qlo8i[:])
            qhi8 = rt.tile([128, 2 * NCH], F32, tag="qhi8")
            nc.vector.tensor_tensor(qhi8[:], qv8[:], qlo8[:], op=OP.subtract)
            nc.vector.tensor_scalar(qhi8[:], qhi8[:], 1.0 / 16.0, None, op0=OP.mult)
            srow = rt.tile([128, 2, NCH], F32, tag="srow")
            nc.vector.tensor_scalar(srow[:], selv, 160.0, None, op0=OP.mult)
            sr2 = rt.tile([128, 2 * NCH], F32, tag="sr2")
            nc.vector.scalar_tensor_tensor(
                sr2[:], qlo8[:], 10.0, srow[:].rearrange("p a b -> p (a b)"),
                op0=OP.mult, op1=OP.add,
            )
            nc.vector.tensor_tensor(sr2[:], sr2[:], qhi8[:], op=OP.add)
            # overflow guard: q >= CAP -> dump row
            ge8 = work.tile([128, 2 * NCH], F32, tag="ge8")
            nc.vector.tensor_scalar(ge8[:], qv8[:], float(CAP), None, op0=OP.is_ge)
            dd8 = work.tile([128, 2 * NCH], F32, tag="dd8")
            nc.vector.tensor_tensor(dd8[:], dumpc[:], sr2[:], op=OP.subtract)
            nc.vector.tensor_tensor(dd8[:], dd8[:], ge8[:], op=OP.mult)
            nc.vector.tensor_tensor(sr2[:], sr2[:], dd8[:], op=OP.add)
            # lanes [128, 8, 64]: [ghi, glo, w, rank, 0...]
            lanes = rt.tile([128, 2 * NCH, 64], F32, tag="lanes")
            nc.vector.memset(lanes[:], 0.0)
            nc.vector.tensor_copy(lanes[:, :, 0], ghic[:])
            nc.vector.tensor_copy(lanes[:, :, 1], gloc[:])
            nc.vector.tensor_copy(
                lanes[:, :, 2].rearrange("p (a b) -> p a b", a=2), wv
            )
            nc.vector.memset(lanes[:, 0:NCH, 3], 1.0)
            nc.vector.memset(lanes[:, NCH:, 3], 2.0)

            # wrap16 of srow over entries i = 128j + p
            shi = rt.tile([128, 2 * NCH], F32, tag="shi")
            slo = rt.tile([128, 2 * NCH], F32, tag="slo")
            svi = work.tile([128, 2 * NCH], I16, tag="svi")
            nc.vector.tensor_copy(svi[:], sr2[:])
            sloi = work.tile([128, 2 * NCH], I16, tag="sloi")
            nc.vector.tensor_scalar(sloi[:], svi[:], 31, None, op0=OP.bitwise_and)
            nc.vector.tensor_copy(slo[:], sloi[:])
            nc.vector.tensor_tensor(shi[:], sr2[:], slo[:], op=OP.subtract)
            nc.vector.tensor_scalar(shi[:], shi[:], 1.0 / 32.0, None, op0=OP.mult)
            rrhs = rt.tile([128, 2, 2 * NCH, E], BF16, tag="rrhs")
            for v in range(E):
                nc.vector.tensor_scalar(
                    rrhs[:, 0, :, v], shi[:], eqv8[:, v : v + 1], None, op0=OP.mult
                )
                nc.vector.tensor_scalar(
                    rrhs[:, 1, :, v], slo[:], eqv8[:, v : v + 1], None, op0=OP.mult
                )
            psr = ps.tile([128, 2, 2 * NCH * E], F32, tag="ps")
            nc.tensor.matmul(
                psr[:].rearrange("p a b -> p (a b)"),
                eqr128[:],
                rrhs[:].rearrange("p a b c -> p (a b c)"),
                start=True, stop=True,
            )
            sidxf = rt.tile([128, 2 * NCH * E], F32, tag="sidxf")
            nc.vector.tensor_scalar(sidxf[:], psr[:, 0, :], 32.0, None, op0=OP.mult)
            nc.vector.tensor_tensor(sidxf[:], sidxf[:], psr[:, 1, :], op=OP.add)
            sidx = rt.tile([128, 2 * NCH * E], I16, tag="sidx")
            nc.vector.tensor_copy(sidx[:], sidxf[:])

            # scatter 1024 full-row entries into the home table
            nc.gpsimd.dma_scatter_add(
                htab[:, :], lanes[:], sidx[:],
                num_idxs=2 * TH, num_idxs_reg=2 * TH, elem_size=64, elem_step=64,
                queue_num=0,
            )

            # ---------- home-side return gather indices (direct, no table) ----------
            # recv row for (rank rk, token chunk c): 128*(q//16) + 16*sel + q%16
            # entries i = 512*rk + 128*c + p, same wrap16 build as the scatter idx
            rvg = rt.tile([128, 2 * NCH], F32, tag="rvg")
            nc.vector.scalar_tensor_tensor(
                rvg[:], selv, 16.0, qlo8[:].rearrange("p (a b) -> p a b", a=2),
                op0=OP.mult, op1=OP.add,
            )
            nc.vector.scalar_tensor_tensor(
                rvg[:], qhi8[:], 128.0, rvg[:], op0=OP.mult, op1=OP.add
            )
            ghi2 = rt.tile([128, 2 * NCH], F32, tag="ghi2")
            glo2 = rt.tile([128, 2 * NCH], F32, tag="glo2")
            gvi = work.tile([128, 2 * NCH], I16, tag="gvi")
            nc.vector.tensor_copy(gvi[:], rvg[:])
            gloi = work.tile([128, 2 * NCH], I16, tag="gloi")
            nc.vector.tensor_scalar(gloi[:], gvi[:], 31, None, op0=OP.bitwise_and)
            nc.vector.tensor_copy(glo2[:], gloi[:])
            nc.vector.tensor_tensor(ghi2[:], rvg[:], glo2[:], op=OP.subtract)
            nc.vector.tensor_scalar(ghi2[:], ghi2[:], 1.0 / 32.0, None, op0=OP.mult)
            rrhs2 = rt.tile([128, 2, 2 * NCH, E], BF16, tag="rrhs2")
            for v in range(E):
                nc.vector.tensor_scalar(
                    rrhs2[:, 0, :, v], ghi2[:], eqv8[:, v : v + 1], None, op0=OP.mult
                )
                nc.vector.tensor_scalar(
                    rrhs2[:, 1, :, v], glo2[:], eqv8[:, v : v + 1], None, op0=OP.mult
                )
            psr3 = ps.tile([128, 2, 2 * NCH * E], F32, tag="ps")
            nc.tensor.matmul(
                psr3[:].rearrange("p a b -> p (a b)"),
                eqr128[:],
                rrhs2[:].rearrange("p a b c -> p (a b c)"),
                start=True, stop=True,
            )
            ridxf = rt.tile([128, 2 * NCH * E], F32, tag="ridxf")
            nc.vector.tensor_scalar(ridxf[:], psr3[:, 0, :], 32.0, None, op0=OP.mult)
            nc.vector.tensor_tensor(ridxf[:], ridxf[:], psr3[:, 1, :], op=OP.add)
            ridxg = rt.tile([128, 2 * NCH * E], I16, tag="ridxg")
            nc.vector.tensor_copy(ridxg[:], ridxf[:])

            # ---------- table A2A (expert-major blocks) ----------
            nc.gpsimd.collective_compute(
                "AllToAll", OP.bypass,
                replica_groups=[list(range(NCORES))],
                ins=[htab[:C, :].rearrange("a b -> (a b)")],
                outs=[htab_x[:].rearrange("a b -> (a b)")],
            )

            # ---------- expert-side reload: [16(u), 8(h), 10(j), 64] ----------
            # loaded in two j-halves to halve SBUF; f = 8*j + h so each half
            # covers a contiguous f-range of 40
            grhs = rt.tile([16, 2, 80], BF16, tag="grhs")
            wspl = rt.tile([16, 2, 80], BF16, tag="wspl")
            rtsrc = htab_x[:].rearrange("(h u j) l -> u h j l", u=16, j=NJ)
            for jh in range(2):
                rtile = rt.tile([16, E, NJ // 2, 64], F32, tag="rtile")
                nc.scalar.dma_start(
                    rtile[:], rtsrc[:, :, 5 * jh : 5 * (jh + 1), :]
                )
                rtv = rtile[:].rearrange("u h j l -> u j h l")
                fsl = slice(40 * jh, 40 * (jh + 1))
                nc.vector.tensor_copy(grhs[:, 0, fsl], rtv[:, :, :, 0])
                nc.vector.tensor_copy(grhs[:, 1, fsl], rtv[:, :, :, 1])
                nc.vector.tensor_copy(wspl[:, 0, fsl], rtv[:, :, :, 2])
                wrem = work.tile([16, 40], F32, tag="wrem")
                nc.vector.tensor_tensor(
                    wrem[:], rtv[:, :, :, 2], wspl[:, 0, fsl], op=OP.subtract
                )
                nc.vector.tensor_copy(wspl[:, 1, fsl], wrem[:])
            psg = psg_pool.tile([128, 2, 80], F32, tag="psg")
            nc.tensor.matmul(
                psg[:].rearrange("p a b -> p (a b)"),
                eqr16[:],
                grhs[:].rearrange("p a b -> p (a b)"),
                start=True, stop=True,
            )
            gxf = rt.tile([128, 80], F32, tag="gxf")
            nc.vector.tensor_scalar(gxf[:], psg[:, 0, :], 32.0, None, op0=OP.mult)
            nc.vector.tensor_tensor(gxf[:], gxf[:], psg[:, 1, :], op=OP.add)
            nc.vector.tensor_scalar(gxf[:], gxf[:], -1.0, None, op0=OP.add)
            nc.vector.tensor_scalar(gxf[:], gxf[:], 0.0, None, op0=OP.max)
            gxidx = rt.tile([128, 80], I16, tag="gxidx")
            nc.vector.tensor_copy(gxidx[:], gxf[:])
            # per-slot weight, slot-major [p = s%128, a = s//128], via wsell matmuls
            psw = psg_pool.tile([128, 2, NJ], F32, tag="psg")
            for b in range(E):
                nc.tensor.matmul(
                    psw[:].rearrange("p a b -> p (a b)"),
                    wsell[:, b, :],
                    wspl[:, :, b::8].rearrange("p a b -> p (a b)"),
                    start=(b == 0), stop=(b == E - 1),
                )
            wslot = rt.tile([128, NJ], F32, tag="wslot")
            nc.vector.tensor_copy(wslot[:], psw[:, 0, :])
            nc.vector.tensor_tensor(wslot[:], wslot[:], psw[:, 1, :], op=OP.add)

            # ---------- x-gathers: 3-buffer ring, 3 prefetched upfront ----------
            def gather_x(g):
                xtf = xgpool.tile([128, KD, GL], BF16, tag=f"xtg{g % 3}")
                nc.gpsimd.dma_gather(
                    xtf[:], xbf.ap(),
                    gxidx[:, 16 * g : 16 * (g + 1)],
                    num_idxs=GL, num_idxs_reg=GL, elem_size=D, transpose=True,
                    queue_num=0,
                )
                return xtf

            xtfs = {g: gather_x(g) for g in range(3)}

            # ---------- FFN over 5 groups of 256 slots + pipelined return ----------
            for g in range(NG):
                xtf = xtfs.pop(g)
                if g + 3 <= NG - 1:
                    xtfs[g + 3] = gather_x(g + 3)
                gt = gtpool.tile([128, KH, GL], BF16, tag="gt")
                for h in range(KH):
                    ph1 = ps.tile([128, GL], F32, tag="ps")
                    for k in range(KD):
                        nc.tensor.matmul(
                            ph1[:], w1sb[:, k, 128 * h : 128 * (h + 1)],
                            xtf[:, k, :],
                            start=(k == 0), stop=(k == KD - 1),
                        )
                    ph0 = ps.tile([128, GL], F32, tag="ps")
                    for k in range(KD):
                        nc.tensor.matmul(
                            ph0[:], w0sb[:, k, 128 * h : 128 * (h + 1)],
                            xtf[:, k, :],
                            start=(k == 0), stop=(k == KD - 1),
                        )
                    sig = work.tile([128, GL], F32, tag="sig")
                    nc.scalar.activation(
                        sig[:], ph1[:], AF.Sigmoid, bias=b1sb[:, h : h + 1]
                    )
                    zb = work.tile([128, GL], F32, tag="zb")
                    nc.vector.tensor_scalar(
                        zb[:], ph1[:], b1sb[:, h : h + 1], None, op0=OP.add
                    )
                    nc.vector.tensor_tensor(zb[:], zb[:], sig[:], op=OP.mult)
                    nc.vector.scalar_tensor_tensor(
                        gt[:, h, :], ph0[:], b0sb[:, h : h + 1], zb[:],
                        op0=OP.add, op1=OP.mult,
                    )
                ysb = ypool.tile([128, GL // 128, D], BF16, tag="ysbg")
                for c2 in range(GL // 128):
                    a = (GL // 128) * g + c2
                    for n in range(D // 512):
                        py = ps.tile([128, 512], F32, tag="ps")
                        for k in range(KH):
                            nc.tensor.matmul(
                                py[:],
                                gt[:, k, 128 * c2 : 128 * (c2 + 1)],
                                w2sb[:, k, 512 * n : 512 * (n + 1)],
                                start=(k == 0), stop=(k == KH - 1),
                            )
                        yb = work.tile([128, 512], F32, tag="yb")
                        nc.vector.tensor_tensor(
                            yb[:], py[:], b2bc[:, 512 * n : 512 * (n + 1)], op=OP.add
                        )
                        nc.vector.tensor_scalar(
                            ysb[:, c2, 512 * n : 512 * (n + 1)], yb[:],
                            wslot[:, a : a + 1], None, op0=OP.mult,
                        )
                    # per-half: contiguous send + A2A + recv + scatter into out
                    kch = (GL // 128) * g + c2
                    nc.sync.dma_start(
                        send[CHL * kch : CHL * (kch + 1), :].rearrange(
                            "(a p) d -> p a d", p=128
                        ),
                        ysb[:, c2 : c2 + 1, :],
                    )
                    nc.gpsimd.collective_compute(
                        "AllToAll", OP.bypass,
                        replica_groups=[list(range(NCORES))],
                        ins=[
                            send[CHL * kch : CHL * (kch + 1), :].rearrange(
                                "a b -> (a b)"
                            )
                        ],
                        outs=[
                            recv[CHL * kch : CHL * (kch + 1), :].rearrange(
                                "a b -> (a b)"
                            )
                        ],
                    )

            # ---------- final: gather both ranks' y rows, add, write out ----------
            y12a = gtpool.tile([128, NCH, D], BF16, tag="gt")
            y12b = gtpool.tile([128, NCH, D], BF16, tag="gt")
            for rk, yt in ((0, y12a), (1, y12b)):
                nc.gpsimd.dma_gather(
                    yt[:], recv[:].opt(),
                    ridxg[:, 32 * rk : 32 * (rk + 1)],
                    num_idxs=TH, num_idxs_reg=TH,
                    elem_size=D, transpose=False, queue_num=0,
                )
            nc.vector.tensor_tensor(y12a[:], y12a[:], y12b[:], op=OP.add)
            outv = out.ap().rearrange("(c p) d -> p c d", p=128)
            nc.sync.dma_start(outv[:], y12a[:])

    nc.compile()
    return nc


def _split_bf16(a):
    hi = a.astype(bf16)
    lo = (a - hi.astype(np.float32)).astype(bf16)
    return hi, lo


def make_in_maps(inputs, gate_w, W0, b0, W1, b1, W2, b2):
    x = np.ascontiguousarray(np.asarray(inputs).reshape(-1, D).astype(np.float32))
    xbf = x.astype(bf16)
    gwT = np.ascontiguousarray(np.asarray(gate_w).astype(np.float32).T)  # [D, E]
    gwhi, gwlo = _split_bf16(gwT)

    p = np.arange(128)
    m = np.arange(128)
    ltri = np.triu(np.ones((128, 128), np.float32)).astype(bf16)
    eqr128 = ((p[:, None] % 16) == (m[None, :] % 16)).astype(np.float32).astype(bf16)
    u = np.arange(16)
    eqr16 = (u[:, None] == (m[None, :] % 16)).astype(np.float32).astype(bf16)
    wsell = np.zeros((16, E, 128), np.float32)
    for b in range(E):
        wsell[:, b, :] = (u[:, None] == (m[None, :] % 16)) & (b == (m[None, :] // 16))
    wsell = wsell.astype(bf16)
    eqv8 = ((p[:, None] // 16) == np.arange(E)[None, :]).astype(np.float32)
    iota8 = np.tile(np.arange(E, dtype=np.float32)[None, None, :], (128, NCH, 1))
    d127 = np.zeros((128, 1), np.float32)
    d127[127, 0] = 1.0
    ones1 = np.ones((1, 128), np.float32)
    # dump rows for scatter overflow: j = 4*rk + c
    dumpc = np.zeros((128, 2 * NCH), np.float32)
    for rk in range(2):
        for c in range(NCH):
            dumpc[:, 4 * rk + c] = C + 2 * (128 * c + p) + rk

    W0 = np.asarray(W0)
    W1 = np.asarray(W1)
    W2 = np.asarray(W2)
    b0 = np.asarray(b0)
    b1 = np.asarray(b1)
    b2 = np.asarray(b2)

    in_maps = []
    for e in range(NCORES):
        xT_own = np.ascontiguousarray(x[e * TH : (e + 1) * TH].T)  # [D, TH]
        xthi, xtlo = _split_bf16(xT_own)
        gid1c = 512 * e + 128 * np.arange(NCH)[None, :] + p[:, None] + 1  # [128, NCH]
        gid1 = np.concatenate([gid1c, gid1c], axis=1)  # [128, 8] cols j = 4rk+c
        mm = {
            "xbf": xbf,
            "xthi": xthi,
            "xtlo": xtlo,
            "gwhi": gwhi,
            "gwlo": gwlo,
            "w0": np.ascontiguousarray(W0[e].astype(bf16)),
            "w1": np.ascontiguousarray(W1[e].astype(bf16)),
            "w2": np.ascontiguousarray(W2[e].astype(bf16)),
            "b0": np.ascontiguousarray(b0[e].astype(np.float32)),
            "b1": np.ascontiguousarray(b1[e].astype(np.float32)),
            "b2": np.ascontiguousarray(b2[e].astype(np.float32)),
            "ltri": ltri,
            "eqr128": eqr128,
            "eqr16": eqr16,
            "wsell": wsell,
            "eqv8": eqv8,
            "iota8": iota8,
            "d127": d127,
            "ones1": ones1,
            "ghic": np.ascontiguousarray((gid1 // 32).astype(np.float32)),
            "gloc": np.ascontiguousarray((gid1 % 32).astype(np.float32)),
            "dumpc": dumpc,
        }
        in_maps.append(mm)
    return in_maps


_NC_CACHE = {}


def get_program(mode="full"):
    if mode not in _NC_CACHE:
        _NC_CACHE[mode] = build_program()
    return _NC_CACHE[mode]


def kernel(**inputs):
    from concourse.bass_utils import run_bass_kernel_spmd

    nc = get_program()
    in_maps = make_in_maps(**inputs)
    res = run_bass_kernel_spmd(nc, in_maps, core_ids=list(range(NCORES)))
    outs = [
        np.asarray(res.results[c]["out"], dtype=np.float32)[:TH]
        for c in range(NCORES)
    ]
    full = np.concatenate(outs, axis=0)
    return full.reshape(np.asarray(inputs["inputs"]).shape)
